# revision 11
# baseline (speedup 1.0000x reference)
"""KAN layer (B-spline + silu) Trainium2 kernel, sharded over d_model on 8 cores.

Math: for the uniform grid the reference's Cox-de-Boor cubic bases collapse to
cardinal B-splines, so per spline
    y_sp(u) = sum_j coef_j * Q(u - j),  u = (x - e0)/h  (extended-grid coord)
with Q the cardinal cubic B-spline. Using Q's truncated-power expansion,
    y_sp(u) = sum_{l=0..11} a_l * relu(u - l)^3,
    a = conv(coef, [1,-4,6,-4,1]/6),
exact for u < 11 and, with u clamped to 11, identically 0 beyond the support
(matching the reference's all-zero bases there). Since the grid is shared
across out_dim, the bases depend only on (d, i, batch) and the layer becomes,
per d, a (b x [i,l]) @ ([i,l] x o) matmul with silu folded in as a 13th term.

Per core: 8 d-slices as 2 groups of 4 (partition = 4d x 32i = 128), per group
12 truncated powers (ACT Square + DVE max/mul) and one Silu, then 13
accumulating PE matmuls with block-diagonal host-prepared coefficient tiles.
"""

import os
import sys

import numpy as np

sys.path.insert(0, "/opt/trn_rl_repo")

_BATCH, _D, _IN, _OUT = 128, 64, 32, 32
_NC = 8            # cores
_DPC = _D // _NC   # d_model slices per core
_NG = 2            # partition groups per core (4 d each)
_NL = 12           # truncated powers

_built_cache = {}
LAST_RESULT = None  # BassKernelResults of the most recent kernel() call


def _build_nc(inv_h, off):
    """Bass program for one core. t_l = max(x*inv_h - (off+l), 0), x*inv_h
    clamped to off+11; grid constants are baked in as immediates."""
    import concourse.bass as bass
    import concourse.tile as tile
    from concourse import bacc, mybir

    f32 = mybir.dt.float32
    Alu = mybir.AluOpType
    Act = mybir.ActivationFunctionType

    nc = bacc.Bacc("TRN2", target_bir_lowering=False, debug=False, num_devices=_NC)
    xt_d = nc.dram_tensor("xt", [_NG, 128, 128], f32, kind="ExternalInput")
    cm_d = nc.dram_tensor("cmat", [_NG, _NL + 1, 128, 128], f32, kind="ExternalInput")
    out_d = nc.dram_tensor("out", [128, _NG * 128], f32, kind="ExternalOutput")

    climit = off + 11.0

    with tile.TileContext(nc) as tc:
        with (
            tc.tile_pool(name="xp", bufs=2) as xp,
            tc.tile_pool(name="xcp", bufs=2) as xcp,
            tc.tile_pool(name="silp", bufs=2) as silp,
            tc.tile_pool(name="tp", bufs=4) as tp,
            tc.tile_pool(name="qp", bufs=4) as qp,
            # every P/C tile gets its own buffer: slot reuse would add a
            # PE-semaphore WAR wait and overflow the 2-wait TT descriptor
            tc.tile_pool(name="pp", bufs=_NG * _NL) as pp,
            tc.tile_pool(name="cp", bufs=_NG * (_NL + 1)) as cp,
            tc.tile_pool(name="outp", bufs=2) as outp,
            tc.tile_pool(name="ps", bufs=2, space=bass.MemorySpace.PSUM) as ps,
        ):
            # bias columns written by DVE so ACT Squares wait on one sem only
            # (the ACT descriptor can't hold two foreign sync-waits)
            bias_t = xp.tile([128, _NL], f32, tag="bias")
            for l in range(_NL):
                nc.vector.memset(bias_t[:, l : l + 1], -(off + l))
            Xs, xcs, sils = [], [], []
            for g in range(_NG):
                X = xp.tile([128, 128], f32)
                nc.sync.dma_start(X[:], xt_d[g])
                Xs.append(X)
            # both Silus up front so ACT visits each function table once
            for g in range(_NG):
                sil = silp.tile([128, 128], f32)
                nc.scalar.activation(sil[:], Xs[g][:], Act.Silu)
                sils.append(sil)
            for g in range(_NG):
                xc = xcp.tile([128, 128], f32)
                nc.vector.tensor_scalar(
                    xc[:], Xs[g][:], inv_h, climit, Alu.mult, Alu.min
                )
                xcs.append(xc)
            for g in range(_NG):
                psum = ps.tile([128, 128], f32)
                for l in range(_NL):
                    c = cp.tile([128, 128], f32)
                    nc.sync.dma_start(c[:], cm_d[g, l])
                    t = tp.tile([128, 128], f32)
                    nc.vector.tensor_scalar(
                        t[:], xcs[g][:], off + l, 0.0, Alu.subtract, Alu.max
                    )
                    q = qp.tile([128, 128], f32)
                    nc.scalar.activation(
                        q[:], xcs[g][:], Act.Square, bias=bias_t[:, l : l + 1]
                    )
                    P = pp.tile([128, 128], f32)
                    nc.vector.tensor_mul(P[:], q[:], t[:])
                    nc.tensor.matmul(psum[:], P[:], c[:], start=(l == 0), stop=False)
                csil = cp.tile([128, 128], f32)
                nc.sync.dma_start(csil[:], cm_d[g, _NL])
                nc.tensor.matmul(psum[:], sils[g][:], csil[:], start=False, stop=True)
                o = outp.tile([128, 128], f32)
                nc.vector.tensor_copy(o[:], psum[:])
                nc.sync.dma_start(out_d[:, g * 128 : (g + 1) * 128], o[:])
    nc.compile()
    return nc


def kernel(x, grid, coef, scale_base, scale_sp, mask, k):
    global LAST_RESULT
    from concourse.bass_utils import run_bass_kernel_spmd

    x = np.asarray(x, dtype=np.float32)
    grid = np.asarray(grid, dtype=np.float32)
    coef64 = np.asarray(coef, dtype=np.float64)
    msp = (np.asarray(mask, dtype=np.float64)
           * np.asarray(scale_sp, dtype=np.float64))
    mb = (np.asarray(mask, dtype=np.float64)
          * np.asarray(scale_base, dtype=np.float64))
    assert int(k) == 3

    g0 = grid[0, 0].astype(np.float64)
    h = float(g0[1] - g0[0])
    e0 = float(g0[0]) - 3.0 * h
    inv_h = 1.0 / h
    off = e0 / h

    # a_l = conv(coef, [1,-4,6,-4,1]/6) along the coefficient axis
    w = np.array([1.0, -4.0, 6.0, -4.0, 1.0]) / 6.0
    a = np.zeros((_D, _D * 16, _NL))  # (64, 1024, 12)
    for j in range(coef64.shape[-1]):
        a[:, :, j : j + 5] += coef64[:, :, j : j + 1] * w

    val = np.empty((_D, 1024, _NL + 1), np.float32)
    val[:, :, :_NL] = (msp[:, :, None] * a).astype(np.float32)
    val[:, :, _NL] = mb.astype(np.float32)
    # (d, s=o*32+i, l~) -> (c, g, ds, i, o, l~)
    v = val.reshape(_D, _OUT, _IN, _NL + 1).transpose(0, 2, 1, 3)
    v = v.reshape(_NC, _NG, 4, _IN, _OUT, _NL + 1)
    cm = np.zeros((_NC, _NG, _NL + 1, 128, 128), np.float32)
    for ds in range(4):
        cm[:, :, :, ds * 32 : (ds + 1) * 32, ds * 32 : (ds + 1) * 32] = (
            v[:, :, ds].transpose(0, 1, 4, 2, 3)
        )

    # x (b, d, i) -> xt (c, g, 4*32, b)
    xt = np.ascontiguousarray(
        x.transpose(1, 2, 0).reshape(_NC, _NG, 128, _BATCH)
    )

    key = (round(inv_h, 12), round(off, 12))
    nc = _built_cache.get(key)
    if nc is None:
        nc = _build_nc(inv_h, off)
        _built_cache[key] = nc

    in_maps = [{"xt": xt[c], "cmat": cm[c]} for c in range(_NC)]
    res = run_bass_kernel_spmd(
        nc,
        in_maps,
        list(range(_NC)),
        trace=os.environ.get("KAN_TRACE") == "1",
    )
    LAST_RESULT = res

    out = np.empty((_BATCH, _D, _OUT), np.float32)
    for c in range(_NC):
        out[:, c * _DPC : (c + 1) * _DPC, :] = res.results[c]["out"].reshape(
            _BATCH, _DPC, _OUT
        )
    return out


# revision 14
# speedup vs baseline: 1.3392x; 1.3392x over previous
"""KAN layer (B-spline + silu) Trainium2 kernel, sharded over d_model on 8 cores.

Math: for the uniform grid the reference's Cox-de-Boor cubic bases collapse to
cardinal B-splines, so per spline
    y_sp(u) = sum_j coef_j * Q(u - j),  u = (x - e0)/h  (extended-grid coord)
with Q the cardinal cubic B-spline. Using Q's truncated-power expansion,
    y_sp(u) = sum_{l=0..11} a_l * relu(u - l)^3,
    a = conv(coef, [1,-4,6,-4,1]/6),
exact for u < 11 and, with u clamped to 11, identically 0 beyond the support
(matching the reference's all-zero bases there). Since the grid is shared
across out_dim, the bases depend only on (d, i, batch) and the layer becomes,
per d, a (b x [i,l]) @ ([i,l] x o) matmul with silu folded in as a 13th term.

Layout per core (8 d-slices as 2 groups of 4): partition = (ds,i) = 128,
free = (group, batch) = 256. Truncated powers P_l = Square(ACT) * max(DVE),
then 13 accumulating PE matmuls per group with block-diagonal coefficient
tiles (C cols sliced per group). Ops are phase-batched so cross-engine waits
collapse to one per phase instead of one per l.
"""

import os
import sys

import numpy as np

sys.path.insert(0, "/opt/trn_rl_repo")

_BATCH, _D, _IN, _OUT = 128, 64, 32, 32
_NC = 8            # cores
_DPC = _D // _NC   # d_model slices per core
_NG = 2            # partition groups per core (4 d each)
_NL = 12           # truncated powers
_F = _NG * _BATCH  # free width of compute tiles

_built_cache = {}
LAST_RESULT = None  # BassKernelResults of the most recent kernel() call


def _build_nc(inv_h, off):
    """Bass program for one core. t_l = max(x*inv_h - (off+l), 0), x*inv_h
    clamped to off+11; grid constants are baked in as immediates."""
    import concourse.bass as bass
    import concourse.tile as tile
    from concourse import bacc, mybir

    f32 = mybir.dt.float32
    Alu = mybir.AluOpType
    Act = mybir.ActivationFunctionType

    nc = bacc.Bacc("TRN2", target_bir_lowering=False, debug=False, num_devices=_NC)
    xt_d = nc.dram_tensor("xt", [128, _F], f32, kind="ExternalInput")
    cm_d = nc.dram_tensor("cmat", [_NL + 1, 128, _F], f32, kind="ExternalInput")
    out_d = nc.dram_tensor("out", [128, _F], f32, kind="ExternalOutput")

    climit = off + 11.0

    with tile.TileContext(nc) as tc:
        with (
            tc.tile_pool(name="xp", bufs=1) as xp,
            tc.tile_pool(name="tp", bufs=_NL) as tp,
            tc.tile_pool(name="qp", bufs=_NL) as qp,
            tc.tile_pool(name="pp", bufs=_NL) as pp,
            tc.tile_pool(name="cp", bufs=_NL + 1) as cp,
            tc.tile_pool(name="ps", bufs=_NG, space=bass.MemorySpace.PSUM) as ps,
        ):
            # bias columns written by DVE so ACT Squares wait on one sem only
            bias_t = xp.tile([128, _NL], f32, tag="bias")
            for l in range(_NL):
                nc.vector.memset(bias_t[:, l : l + 1], -(off + l))

            X = xp.tile([128, _F], f32, tag="x")
            nc.sync.dma_start(X[:], xt_d[:])
            Cs = []
            for l in range(_NL + 1):
                c = cp.tile([128, _F], f32)
                nc.sync.dma_start(c[:], cm_d[l])
                Cs.append(c)

            sil = xp.tile([128, _F], f32, tag="sil")
            nc.scalar.activation(sil[:], X[:], Act.Silu)
            xc = xp.tile([128, _F], f32, tag="xc")
            nc.vector.tensor_scalar(xc[:], X[:], inv_h, climit, Alu.mult, Alu.min)

            qs, ts = [], []
            for l in range(_NL):
                q = qp.tile([128, _F], f32)
                nc.scalar.activation(q[:], xc[:], Act.Square, bias=bias_t[:, l : l + 1])
                qs.append(q)
            for l in range(_NL):
                t = tp.tile([128, _F], f32)
                nc.vector.tensor_scalar(
                    t[:], xc[:], off + l, 0.0, Alu.subtract, Alu.max
                )
                ts.append(t)
            Ps = []
            for l in range(_NL):
                P = pp.tile([128, _F], f32)
                nc.vector.tensor_mul(P[:], qs[l][:], ts[l][:])
                Ps.append(P)

            psums = [
                ps.tile([128, _BATCH], f32, name=f"psum{g}", tag=f"psum{g}")
                for g in range(_NG)
            ]
            for g in range(_NG):
                gs = slice(g * _BATCH, (g + 1) * _BATCH)
                for l in range(_NL):
                    nc.tensor.matmul(
                        psums[g][:], Ps[l][:, gs], Cs[l][:, gs],
                        start=(l == 0), stop=False,
                    )
                nc.tensor.matmul(
                    psums[g][:], sil[:, gs], Cs[_NL][:, gs],
                    start=False, stop=True,
                )
            o = xp.tile([128, _F], f32, tag="out")
            for g in range(_NG):
                nc.vector.tensor_copy(o[:, g * _BATCH : (g + 1) * _BATCH], psums[g][:])
            nc.sync.dma_start(out_d[:], o[:])
    nc.compile()
    return nc


def kernel(x, grid, coef, scale_base, scale_sp, mask, k):
    global LAST_RESULT
    from concourse.bass_utils import run_bass_kernel_spmd

    x = np.asarray(x, dtype=np.float32)
    grid = np.asarray(grid, dtype=np.float32)
    coef64 = np.asarray(coef, dtype=np.float64)
    msp = (np.asarray(mask, dtype=np.float64)
           * np.asarray(scale_sp, dtype=np.float64))
    mb = (np.asarray(mask, dtype=np.float64)
          * np.asarray(scale_base, dtype=np.float64))
    assert int(k) == 3

    g0 = grid[0, 0].astype(np.float64)
    h = float(g0[1] - g0[0])
    e0 = float(g0[0]) - 3.0 * h
    inv_h = 1.0 / h
    off = e0 / h

    # a_l = conv(coef, [1,-4,6,-4,1]/6) along the coefficient axis
    w = np.array([1.0, -4.0, 6.0, -4.0, 1.0]) / 6.0
    a = np.zeros((_D, _D * 16, _NL))  # (64, 1024, 12)
    for j in range(coef64.shape[-1]):
        a[:, :, j : j + 5] += coef64[:, :, j : j + 1] * w

    val = np.empty((_D, 1024, _NL + 1), np.float32)
    val[:, :, :_NL] = (msp[:, :, None] * a).astype(np.float32)
    val[:, :, _NL] = mb.astype(np.float32)
    # (d, s=o*32+i, l~) -> (c, g, ds, i, o, l~)
    v = val.reshape(_D, _OUT, _IN, _NL + 1).transpose(0, 2, 1, 3)
    v = v.reshape(_NC, _NG, 4, _IN, _OUT, _NL + 1)
    cm = np.zeros((_NC, _NG, _NL + 1, 128, 128), np.float32)
    for ds in range(4):
        cm[:, :, :, ds * 32 : (ds + 1) * 32, ds * 32 : (ds + 1) * 32] = (
            v[:, :, ds].transpose(0, 1, 4, 2, 3)
        )
    # -> (c, l~, row, (g, col)) so one DMA per l~ loads both groups
    cm = np.ascontiguousarray(cm.transpose(0, 2, 3, 1, 4)).reshape(
        _NC, _NL + 1, 128, _F
    )

    # x (b, d, i) -> xt (c, (ds, i), (g, b))
    xt = np.ascontiguousarray(
        x.transpose(1, 2, 0)               # (d, i, b)
        .reshape(_NC, _NG, 4, _IN, _BATCH)  # (c, g, ds, i, b)
        .transpose(0, 2, 3, 1, 4)           # (c, ds, i, g, b)
        .reshape(_NC, 128, _F)
    )

    key = (round(inv_h, 12), round(off, 12))
    nc = _built_cache.get(key)
    if nc is None:
        nc = _build_nc(inv_h, off)
        _built_cache[key] = nc

    in_maps = [{"xt": xt[c], "cmat": cm[c]} for c in range(_NC)]
    res = run_bass_kernel_spmd(
        nc,
        in_maps,
        list(range(_NC)),
        trace=os.environ.get("KAN_TRACE") == "1",
    )
    LAST_RESULT = res

    out = np.empty((_BATCH, _D, _OUT), np.float32)
    for c in range(_NC):
        # device out cols are (g, ds, o); rows are b
        out[:, c * _DPC : (c + 1) * _DPC, :] = res.results[c]["out"].reshape(
            _BATCH, _DPC, _OUT
        )
    return out
